# revision 16
# baseline (speedup 1.0000x reference)
"""Trainium2 Bass kernel for nn_Net_79937931313639 (gnn_message_passing).

Strategy
--------
Host side (sharding/planning only): recursive-coordinate-bisection sort of the
16384 points, per-tile candidate granule lists built from sound kNN-radius
upper bounds, load-balanced tile->core assignment, and data routing (gather /
layout shuffling) between device launches.

Device side (all numerics): a sequence of SPMD launches on 8 NeuronCores:
  1. knn       exact top-20 (and top-10 prefix) by d2 among planned candidates
  2. stencil   local PCA (branchless Jacobi), tangent frame, weighted LS fit
               -> Gx, Gy stencils; v_init = grad(pos)
  3. x0        layer-0 scalar path (+ per-edge transport coeffs alpha/beta)
  4. v0        layer-0 vector path
  5. x1        layer-1 scalar path
  6. v1        layer-1 vector path
  7. x2        layer-2 scalar path
Outputs are un-permuted and returned as the reference's (x0, x1, x2) tuple.
"""

import math
import numpy as np

import concourse.bass as bass
import concourse.mybir as mybir
from concourse import bacc
from concourse.tile import TileContext
from concourse.bass_utils import run_bass_kernel_spmd

F32 = mybir.dt.float32
U32 = mybir.dt.uint32
U16 = mybir.dt.uint16
I16 = mybir.dt.int16
AF = mybir.ActivationFunctionType
OP = mybir.AluOpType

K_GRAD, K_NORMAL = 20, 10
GRAD_REG, KERNEL_WIDTH, EPS = 1e-3, 1.0, 1e-8
NPTS, TILE = 16384, 128
NTILES = NPTS // TILE
NCORES = 8
TPC = NTILES // NCORES          # own tiles per core (layer launches)
OWN = NPTS // NCORES            # own points per core
GQ, GC = 16, 16                 # plan granularities
SEL_CHUNK = 128                 # selection chunk for max8
KSEL = 24                       # select top-24 (>=20)

_TRACE = {"on": False, "dir": None}


# ------------------------------------------------------------------
# host planning
# ------------------------------------------------------------------

def rcb_perm(pos, depth=10):
    idx = np.arange(len(pos))

    def rec(ids, d):
        if d == 0:
            return [ids]
        P = pos[ids]
        dim = int(np.argmax(P.max(0) - P.min(0)))
        order = ids[np.argsort(P[:, dim], kind="stable")]
        h = len(order) // 2
        return rec(order[:h], d - 1) + rec(order[h:], d - 1)

    return np.concatenate(rec(idx, depth))


def build_plan(pos_s):
    """Sound candidate granule lists per query tile of 128 sorted points."""
    N = NPTS
    f = np.float32
    # r20 upper bound via sorted-window top-20 (incl self), then refine once
    idx = np.arange(N)
    offs = np.arange(-64, 65)
    win = np.clip(idx[:, None] + offs[None, :], 0, N - 1)
    d2w = ((pos_s[win] - pos_s[:, None, :]) ** 2).sum(-1)
    r20 = np.sqrt(np.sort(d2w, 1)[:, K_GRAD - 1]).astype(f)

    ncnd = N // GC
    c = pos_s.reshape(ncnd, GC, 3)
    clo, chi = c.min(1), c.max(1)
    nq = N // GQ
    q = pos_s.reshape(nq, GQ, 3)
    qlo, qhi = q.min(1), q.max(1)
    sq = (pos_s * pos_s).sum(1)

    def granules_for(r20b):
        rq = r20b.reshape(nq, GQ).max(1)
        lists = []
        for t in range(NTILES):
            sel = set()
            for g in range(t * (TILE // GQ), (t + 1) * (TILE // GQ)):
                gap = np.maximum(0, np.maximum(qlo[g] - chi, clo - qhi[g]))
                boxd = np.sqrt((gap ** 2).sum(-1))
                sel.update(np.where(boxd <= rq[g] + 1e-6)[0].tolist())
            lists.append(np.array(sorted(sel), dtype=np.int64))
        return lists

    lists = granules_for(r20)
    # refine r20 with exact top-20 within current candidates (sound), rebuild
    newr = np.empty_like(r20)
    for t in range(NTILES):
        cols = (lists[t][:, None] * GC + np.arange(GC)).ravel()
        pts = slice(t * TILE, (t + 1) * TILE)
        d2 = sq[pts, None] + sq[cols][None, :] - 2.0 * pos_s[pts] @ pos_s[cols].T
        newr[pts] = np.sqrt(np.maximum(np.sort(d2, 1)[:, K_GRAD - 1], 0))
    r20 = np.minimum(r20, newr.astype(f))
    lists = granules_for(r20)
    return lists


def knn_host_check_rounds(keys, cand):
    """Max members of the true top-KSEL in any SEL_CHUNK chunk -> rounds."""
    order = np.argsort(-keys, axis=1, kind="stable")[:, :KSEL]
    chunks = order // SEL_CHUNK
    need = 1
    for r in range(keys.shape[0]):
        cnt = np.bincount(chunks[r], minlength=1)
        need = max(need, int(math.ceil(cnt.max() / 8)))
    return need


# ------------------------------------------------------------------
# launch 1: knn
# ------------------------------------------------------------------

def build_knn_launch(tile_cands, tile_rounds):
    """tile_cands: per tile-slot candidate counts (multiples of 512).
    tile_rounds: per tile-slot rounds (1..3) for chunked max8.
    Outputs per tile: chunk-local winner indices (widx), merge positions
    (mpos), merge values (mval). Host composes final slots.
    """
    ntile = len(tile_cands)
    offs = np.concatenate([[0], np.cumsum(tile_cands)]).astype(int)
    ctot = int(offs[-1])
    nwin_max = max(8 * (int(c) // SEL_CHUNK) * int(r)
                   for c, r in zip(tile_cands, tile_rounds))

    nc = bacc.Bacc("TRN2", target_bir_lowering=False)
    qd = nc.dram_tensor("qd", [4, ntile * TILE], F32, kind="ExternalInput")
    cs = nc.dram_tensor("cs", [4, ctot], F32, kind="ExternalInput")
    o_widx = nc.dram_tensor("widx", [ntile * TILE, nwin_max], U16, kind="ExternalOutput")
    o_mpos = nc.dram_tensor("mpos", [ntile * TILE, KSEL], U16, kind="ExternalOutput")
    o_mval = nc.dram_tensor("mval", [ntile * TILE, KSEL], F32, kind="ExternalOutput")

    with TileContext(nc) as tc:
        with tc.tile_pool(name="const", bufs=1) as cpool, \
             tc.tile_pool(name="slab", bufs=3) as spool, \
             tc.tile_pool(name="work", bufs=2) as pool, \
             tc.tile_pool(name="psum", bufs=2, space="PSUM") as psum:
            qd_s = cpool.tile([4, ntile * TILE], F32)
            nc.sync.dma_start(qd_s[:], qd[:])

            SEG = 2048
            for t in range(ntile):
                cand = int(tile_cands[t])
                co = int(offs[t])
                nch = cand // SEL_CHUNK
                rounds = int(tile_rounds[t])
                nwin = 8 * nch * rounds

                win = pool.tile([TILE, nwin], F32, tag="win")
                widx = pool.tile([TILE, nwin_max], U16, tag="widx")

                for g0 in range(0, cand, SEG):
                    gl = min(SEG, cand - g0)
                    cs_s = spool.tile([4, SEG], F32, tag="cs")
                    nc.sync.dma_start(cs_s[:, :gl], cs[:, co + g0: co + g0 + gl])
                    keys = pool.tile([TILE, SEG], F32, tag="keys")
                    for s0 in range(0, gl, 512):
                        sl = min(512, gl - s0)
                        kp = psum.tile([TILE, sl], F32, tag="kp")
                        nc.tensor.matmul(kp[:], qd_s[:, t * TILE:(t + 1) * TILE],
                                         cs_s[:, s0: s0 + sl],
                                         start=True, stop=True)
                        nc.scalar.copy(keys[:, s0:s0 + sl], kp[:])
                    for r in range(rounds):
                        for cl in range(gl // SEL_CHUNK):
                            ci = g0 // SEL_CHUNK + cl
                            wsl = slice((r * nch + ci) * 8, (r * nch + ci) * 8 + 8)
                            kslc = keys[:, cl * SEL_CHUNK:(cl + 1) * SEL_CHUNK]
                            nc.vector.max(out=win[:, wsl], in_=kslc)
                            nc.vector.max_index(out=widx[:, wsl], in_max=win[:, wsl],
                                                in_values=kslc)
                        if r + 1 < rounds:
                            for cl in range(gl // SEL_CHUNK):
                                ci = g0 // SEL_CHUNK + cl
                                wsl = slice((r * nch + ci) * 8, (r * nch + ci) * 8 + 8)
                                kslc = keys[:, cl * SEL_CHUNK:(cl + 1) * SEL_CHUNK]
                                nc.vector.match_replace(out=kslc, in_to_replace=win[:, wsl],
                                                        in_values=kslc, imm_value=-1e30)

                mval = pool.tile([TILE, KSEL], F32, tag="mval")
                mpos = pool.tile([TILE, KSEL], U16, tag="mpos")
                for r in range(3):
                    vsl = slice(r * 8, r * 8 + 8)
                    nc.vector.max(out=mval[:, vsl], in_=win[:])
                    nc.vector.max_index(out=mpos[:, vsl], in_max=mval[:, vsl],
                                        in_values=win[:])
                    if r < 2:
                        nc.vector.match_replace(out=win[:], in_to_replace=mval[:, vsl],
                                                in_values=win[:], imm_value=-1e30)
                if nwin < nwin_max:
                    nc.vector.memset(widx[:, nwin:], 0)
                nc.sync.dma_start(o_widx[t * TILE:(t + 1) * TILE, :], widx[:])
                nc.sync.dma_start(o_mpos[t * TILE:(t + 1) * TILE, :], mpos[:])
                nc.sync.dma_start(o_mval[t * TILE:(t + 1) * TILE, :], mval[:])
    nc.compile()
    return nc, nwin_max


# ------------------------------------------------------------------
# runner helper
# ------------------------------------------------------------------

def run_launch(nc, in_maps, name="launch"):
    kw = {}
    if _TRACE["on"]:
        kw = dict(trace=True, tmpdir=f"{_TRACE['dir']}/{name}")
    res = run_bass_kernel_spmd(nc, in_maps, core_ids=list(range(NCORES)), **kw)
    if _TRACE["on"] and res.exec_time_ns:
        print(f"[trace] {name}: exec {res.exec_time_ns} ns")
    return res


# ------------------------------------------------------------------
# phase drivers (host planning + assembly)
# ------------------------------------------------------------------

def plan_and_knn(pos):
    """Returns (perm, nbr_sorted [N,20] global sorted-space ids)."""
    perm = rcb_perm(pos.astype(np.float32))
    pos_s = pos[perm].astype(np.float32)
    sq = (pos_s * pos_s).sum(1).astype(np.float32)

    glists = build_plan(pos_s)

    # per-tile candidate columns, padded to multiple of 512 with distinct granules
    tile_cols = []
    tile_rounds = []
    ncnd = NPTS // GC
    for t in range(NTILES):
        g = list(glists[t])
        need = int(math.ceil(len(g) * GC / 512.0) * 512) // GC
        if need > ncnd:
            need = ncnd
        extra = [u for u in range(ncnd) if u not in set(g)]
        g = (g + extra[: need - len(g)])[:need]
        cols = (np.array(g)[:, None] * GC + np.arange(GC)[None, :]).ravel()
        tile_cols.append(cols)
        # host-side exactness check for chunked selection rounds
        pts = slice(t * TILE, (t + 1) * TILE)
        keys = (2.0 * pos_s[pts] @ pos_s[cols].T - sq[cols][None, :]).astype(np.float32)
        tile_rounds.append(knn_host_check_rounds(keys, len(cols)))

    cand_sizes = np.array([len(c) for c in tile_cols])

    # shuffle candidates within each tile so true top-K spreads across chunks
    # (otherwise spatially-ordered slabs concentrate winners in one chunk and
    # force extra max8 rounds)
    rng = np.random.default_rng(12345)
    for t in range(NTILES):
        p = rng.permutation(len(tile_cols[t]))
        tile_cols[t] = tile_cols[t][p]
        pts = slice(t * TILE, (t + 1) * TILE)
        keys = (2.0 * pos_s[pts] @ pos_s[tile_cols[t]].T
                - sq[tile_cols[t]][None, :]).astype(np.float32)
        tile_rounds[t] = knn_host_check_rounds(keys, len(tile_cols[t]))

    # rank-matched dealing: sort tiles by size desc, slot i <- ranks [8i, 8i+8)
    # so per-slot max across cores is tight and per-core sums are balanced
    order = np.argsort(-cand_sizes, kind="stable")
    nslot = NTILES // NCORES
    core_tiles = [[] for _ in range(NCORES)]
    for i in range(nslot):
        for c in range(NCORES):
            core_tiles[c].append(int(order[i * NCORES + c]))
    ntile_max = nslot

    slot_cands = [max(cand_sizes[ct[i]] for ct in core_tiles) for i in range(ntile_max)]
    slot_rounds = [max(tile_rounds[ct[i]] for ct in core_tiles) for i in range(ntile_max)]

    nc, nwin_max = build_knn_launch(slot_cands, slot_rounds)

    offs = np.concatenate([[0], np.cumsum(slot_cands)]).astype(int)
    ctot = int(offs[-1])

    in_maps = []
    core_cols = []  # per core per slot: candidate cols array
    for c in range(NCORES):
        qd = np.zeros((4, ntile_max * TILE), np.float32)
        cslab = np.zeros((4, ctot), np.float32)
        cols_per_slot = []
        for i, t in enumerate(core_tiles[c]):
            pts = slice(t * TILE, (t + 1) * TILE)
            qd[0:3, i * TILE:(i + 1) * TILE] = 2.0 * pos_s[pts].T
            qd[3, i * TILE:(i + 1) * TILE] = -1.0
            cols = tile_cols[t]
            cols_per_slot.append(cols)
            sl = slice(int(offs[i]), int(offs[i]) + len(cols))
            cslab[0:3, sl] = pos_s[cols].T
            cslab[3, sl] = sq[cols]
            # pad region beyond len(cols) stays zero; key = -sq_j = 0 for
            # zero-pad -> could beat real keys! fill pad with +inf distance
            pad = slice(int(offs[i]) + len(cols), int(offs[i + 1]))
            cslab[3, pad] = 1e30
        in_maps.append(dict(qd=qd, cs=cslab))
        core_cols.append(cols_per_slot)

    res = run_launch(nc, in_maps, "knn")

    nbr = np.zeros((NPTS, K_GRAD), np.int64)
    for c in range(NCORES):
        widx = res.results[c]["widx"]
        mpos = res.results[c]["mpos"].astype(np.int64)
        for i, t in enumerate(core_tiles[c]):
            if core_tiles[c].index(t) != i:
                continue  # padded repeat
            cand = slot_cands[i]
            nch = cand // SEL_CHUNK
            rows = slice(i * TILE, (i + 1) * TILE)
            mp = mpos[rows, :K_GRAD]
            wi = np.take_along_axis(widx[rows].astype(np.int64), mp, 1)
            chunk = (mp % (nch * 8)) // 8
            slots = wi + chunk * SEL_CHUNK
            cols = core_cols[c][i]
            nbr[t * TILE:(t + 1) * TILE] = cols[slots]
    return perm, nbr




# ------------------------------------------------------------------
# launch 2: stencils (basis + LS fit + v_init)
# ------------------------------------------------------------------

def build_stencil_launch():
    """Per core: 16 own tiles. Point-major layouts [128, 16, ...].
    in:  pi [128,16,3], pn [128,16,3,20]
    out: bas [128,16,6] (xb,yb), gx/gy [128,16,20], vin [128,16,6]
    """
    nc = bacc.Bacc("TRN2", target_bir_lowering=False)
    d_pi = nc.dram_tensor("pi", [TILE, TPC, 3], F32, kind="ExternalInput")
    d_pn = nc.dram_tensor("pn", [TILE, TPC, 3, K_GRAD], F32, kind="ExternalInput")
    d_bas = nc.dram_tensor("bas", [TILE, TPC, 6], F32, kind="ExternalOutput")
    d_gx = nc.dram_tensor("gx", [TILE, TPC, K_GRAD], F32, kind="ExternalOutput")
    d_gy = nc.dram_tensor("gy", [TILE, TPC, K_GRAD], F32, kind="ExternalOutput")
    d_vin = nc.dram_tensor("vin", [TILE, TPC, 6], F32, kind="ExternalOutput")

    V = nc.vector
    S = nc.scalar
    T = TPC

    with TileContext(nc) as tc:
        with tc.tile_pool(name="sb", bufs=1) as pool:
            eps_col = pool.tile([TILE, 1], F32)
            V.memset(eps_col[:], float(EPS))
            pi = pool.tile([TILE, T, 3], F32)
            pn = pool.tile([TILE, T, 3, K_GRAD], F32)
            nc.sync.dma_start(pi[:], d_pi[:])
            nc.sync.dma_start(pn[:], d_pn[:])

            def bc(ap, shape):
                # broadcast a view to shape by appending step-0 dims
                while ap.ndim < len(shape):
                    ap = ap.unsqueeze(ap.ndim)
                return ap.broadcast_to(shape)

            rel = pool.tile([TILE, T, 3, K_GRAD], F32)
            V.tensor_sub(rel[:], pn[:], bc(pi[:], (TILE, T, 3, K_GRAD)))

            # --- covariance over first 10 neighbors ---
            # channel order (xx, yy, zz, xy, xz, yz)
            covp = pool.tile([TILE, T, 6, K_NORMAL], F32)
            r10 = rel[:, :, :, 0:K_NORMAL]
            V.tensor_mul(covp[:, :, 0:3, :], r10, r10)
            V.tensor_mul(covp[:, :, 3, :], rel[:, :, 0, 0:K_NORMAL], rel[:, :, 1, 0:K_NORMAL])
            V.tensor_mul(covp[:, :, 4, :], rel[:, :, 0, 0:K_NORMAL], rel[:, :, 2, 0:K_NORMAL])
            V.tensor_mul(covp[:, :, 5, :], rel[:, :, 1, 0:K_NORMAL], rel[:, :, 2, 0:K_NORMAL])
            cov6 = pool.tile([TILE, T, 6], F32)
            V.tensor_reduce(cov6[:], covp[:], axis=mybir.AxisListType.X, op=OP.add)

            # --- branchless cyclic Jacobi, 4 sweeps ---
            # cov6 channels: a00=0, a11=1, a22=2, a01=3, a02=4, a12=5
            amap = {(0, 0): 0, (1, 1): 1, (2, 2): 2, (0, 1): 3, (1, 0): 3,
                    (0, 2): 4, (2, 0): 4, (1, 2): 5, (2, 1): 5}
            Vm = pool.tile([TILE, T, 3, 3], F32)  # column-major: [col, d]
            V.memset(Vm[:], 0.0)
            for c in range(3):
                V.memset(Vm[:, :, c, c], 1.0)

            sc = {n: pool.tile([TILE, T], F32, tag=f"jsc{n}", name=f"jsc_{n}") for n in
                  "d dd apq2 r2 r absd den rden msk sgn t tnum t2 c2d rc2 c s tmp1 tmp2 tmp3".split()}

            def A(i, j):
                return cov6[:, :, amap[(i, j)]]

            for sweep in range(4):
                for (p, q) in [(0, 1), (0, 2), (1, 2)]:
                    m = 3 - p - q
                    V.tensor_sub(sc["d"][:], A(q, q), A(p, p))
                    V.tensor_mul(sc["dd"][:], sc["d"][:], sc["d"][:])
                    V.tensor_mul(sc["apq2"][:], A(p, q), A(p, q))
                    V.scalar_tensor_tensor(sc["r2"][:], sc["apq2"][:], 4.0, sc["dd"][:],
                                           op0=OP.mult, op1=OP.add)
                    S.activation(sc["r"][:], sc["r2"][:], AF.Sqrt)
                    S.activation(sc["absd"][:], sc["d"][:], AF.Abs)
                    V.scalar_tensor_tensor(sc["den"][:], sc["absd"][:], 1e-30, sc["r"][:],
                                           op0=OP.add, op1=OP.add)
                    V.reciprocal(sc["rden"][:], sc["den"][:])
                    V.tensor_scalar(sc["msk"][:], sc["d"][:], 0.0, scalar2=None, op0=OP.is_ge)
                    V.tensor_scalar(sc["sgn"][:], sc["msk"][:], 2.0, scalar2=-1.0,
                                    op0=OP.mult, op1=OP.add)
                    V.scalar_tensor_tensor(sc["tnum"][:], A(p, q), 2.0, sc["sgn"][:],
                                           op0=OP.mult, op1=OP.mult)
                    V.tensor_mul(sc["t"][:], sc["tnum"][:], sc["rden"][:])
                    V.tensor_mul(sc["t2"][:], sc["t"][:], sc["t"][:])
                    V.tensor_scalar(sc["c2d"][:], sc["t2"][:], 1.0, scalar2=None, op0=OP.add)
                    V.reciprocal(sc["rc2"][:], sc["c2d"][:])
                    S.activation(sc["c"][:], sc["rc2"][:], AF.Sqrt)
                    V.tensor_mul(sc["s"][:], sc["t"][:], sc["c"][:])
                    # diag updates: app -= t*apq ; aqq += t*apq ; apq = 0
                    V.tensor_mul(sc["tmp1"][:], sc["t"][:], A(p, q))
                    V.tensor_sub(A(p, p), A(p, p), sc["tmp1"][:])
                    V.tensor_add(A(q, q), A(q, q), sc["tmp1"][:])
                    V.memset(A(p, q), 0.0)
                    # off-diag row m: amp' = c*amp - s*amq ; amq' = s*amp + c*amq
                    V.tensor_mul(sc["tmp1"][:], sc["c"][:], A(m, p))
                    V.tensor_mul(sc["tmp2"][:], sc["s"][:], A(m, q))
                    V.tensor_mul(sc["tmp3"][:], sc["s"][:], A(m, p))
                    V.tensor_sub(A(m, p), sc["tmp1"][:], sc["tmp2"][:])
                    V.tensor_mul(sc["tmp1"][:], sc["c"][:], A(m, q))
                    V.tensor_add(A(m, q), sc["tmp3"][:], sc["tmp1"][:])
                    # V columns p,q (all 3 rows at once)
                    cb = bc(sc["c"][:], (TILE, T, 3))
                    sb_ = bc(sc["s"][:], (TILE, T, 3))
                    vp = Vm[:, :, p, :]
                    vq = Vm[:, :, q, :]
                    t1_3 = pool.tile([TILE, T, 3], F32, tag="jv1")
                    t2_3 = pool.tile([TILE, T, 3], F32, tag="jv2")
                    V.tensor_mul(t1_3[:], vp, cb)
                    V.tensor_mul(t2_3[:], vq, sb_)
                    V.tensor_sub(t1_3[:], t1_3[:], t2_3[:])   # new vp
                    V.tensor_mul(t2_3[:], vp, sb_)
                    V.tensor_copy(vp, t1_3[:])
                    V.tensor_mul(t1_3[:], vq, cb)
                    V.tensor_add(vq, t2_3[:], t1_3[:])

            # --- pick eigvec of smallest eigenvalue ---
            ev = [cov6[:, :, 0], cov6[:, :, 1], cov6[:, :, 2]]
            m01 = pool.tile([TILE, T], U32, name="mask1")
            m2 = pool.tile([TILE, T], U32, name="mask2")
            V.tensor_tensor(out=m01[:], in0=ev[0], in1=ev[1], op=OP.is_le)
            emin = sc["tmp1"]
            V.select(emin[:], m01[:], ev[0], ev[1])
            nrm = pool.tile([TILE, T, 3], F32)
            t1_3 = pool.tile([TILE, T, 3], F32, tag="jv1")
            V.select(nrm[:], bc(m01[:], (TILE, T, 3)), Vm[:, :, 0, :], Vm[:, :, 1, :])
            V.tensor_tensor(out=m2[:], in0=emin[:], in1=ev[2], op=OP.is_le)
            V.tensor_copy(t1_3[:], nrm[:])
            V.select(nrm[:], bc(m2[:], (TILE, T, 3)), t1_3[:], Vm[:, :, 2, :])

            # orient: flip if dot(normal, pos) < 0
            prod3 = pool.tile([TILE, T, 3], F32, tag="jv2")
            V.tensor_mul(prod3[:], nrm[:], pi[:])
            dotn = sc["tmp3"]
            V.tensor_reduce(dotn[:], prod3[:], axis=mybir.AxisListType.X, op=OP.add)
            V.tensor_scalar(m01[:], dotn[:], 0.0, scalar2=None, op0=OP.is_lt)
            V.tensor_scalar_mul(prod3[:], nrm[:], -1.0)
            V.copy_predicated(nrm[:], bc(m01[:], (TILE, T, 3)), prod3[:])
            # normalize
            V.tensor_mul(prod3[:], nrm[:], nrm[:])
            nn = sc["d"]
            V.tensor_reduce(nn[:], prod3[:], axis=mybir.AxisListType.X, op=OP.add)
            S.activation(sc["r"][:], nn[:], AF.Sqrt)
            V.reciprocal(sc["rden"][:], sc["r"][:])
            V.tensor_mul(nrm[:], nrm[:], bc(sc["rden"][:], (TILE, T, 3)))

            # --- tangent frame ---
            bas = pool.tile([TILE, T, 6], F32)
            t1f = pool.tile([TILE, T, 3], F32, tag="fr1")
            t2f = pool.tile([TILE, T, 3], F32, tag="fr2")
            # t1 = (0, nz, -ny); t2 = (-nz, 0, nx)
            V.memset(t1f[:, :, 0], 0.0)
            V.tensor_copy(t1f[:, :, 1], nrm[:, :, 2])
            V.tensor_scalar_mul(t1f[:, :, 2], nrm[:, :, 1], -1.0)
            V.memset(t2f[:, :, 1], 0.0)
            V.tensor_scalar_mul(t2f[:, :, 0], nrm[:, :, 2], -1.0)
            V.tensor_copy(t2f[:, :, 2], nrm[:, :, 0])
            n1 = sc["dd"]
            n2 = sc["apq2"]
            V.tensor_mul(prod3[:], t1f[:], t1f[:])
            V.tensor_reduce(n1[:], prod3[:], axis=mybir.AxisListType.X, op=OP.add)
            V.tensor_mul(prod3[:], t2f[:], t2f[:])
            V.tensor_reduce(n2[:], prod3[:], axis=mybir.AxisListType.X, op=OP.add)
            V.tensor_tensor(out=m01[:], in0=n1[:], in1=n2[:], op=OP.is_gt)
            xb = bas[:, :, 0:3]
            V.select(xb, bc(m01[:], (TILE, T, 3)), t1f[:], t2f[:])
            V.tensor_mul(prod3[:], xb, xb)
            V.tensor_reduce(nn[:], prod3[:], axis=mybir.AxisListType.X, op=OP.add)
            S.activation(sc["r"][:], nn[:], AF.Sqrt)
            V.reciprocal(sc["rden"][:], sc["r"][:])
            V.tensor_mul(xb, xb, bc(sc["rden"][:], (TILE, T, 3)))
            yb = bas[:, :, 3:6]
            # yb = cross(nrm, xb)
            for dch in range(3):
                i1, i2 = (dch + 1) % 3, (dch + 2) % 3
                V.tensor_mul(sc["tmp1"][:], nrm[:, :, i1], xb[:, :, i2])
                V.tensor_mul(sc["tmp2"][:], nrm[:, :, i2], xb[:, :, i1])
                V.tensor_sub(yb[:, :, dch], sc["tmp1"][:], sc["tmp2"][:])
            nc.sync.dma_start(d_bas[:], bas[:])

            # --- weighted LS fit ---
            u = pool.tile([TILE, T, K_GRAD], F32)
            vv = pool.tile([TILE, T, K_GRAD], F32)
            tmpk = pool.tile([TILE, T, K_GRAD], F32)
            for (dst, b0) in ((u, 0), (vv, 3)):
                V.tensor_mul(dst[:], rel[:, :, 0, :], bc(bas[:, :, b0 + 0], (TILE, T, K_GRAD)))
                V.tensor_mul(tmpk[:], rel[:, :, 1, :], bc(bas[:, :, b0 + 1], (TILE, T, K_GRAD)))
                V.tensor_add(dst[:], dst[:], tmpk[:])
                V.tensor_mul(tmpk[:], rel[:, :, 2, :], bc(bas[:, :, b0 + 2], (TILE, T, K_GRAD)))
                V.tensor_add(dst[:], dst[:], tmpk[:])

            relsq = pool.tile([TILE, T, 3, K_GRAD], F32)
            V.tensor_mul(relsq[:], rel[:], rel[:])
            ssq = pool.tile([TILE, T, K_GRAD], F32)
            V.tensor_reduce(ssq[:], relsq[:].transpose([0, 1, 3, 2]),
                            axis=mybir.AxisListType.X, op=OP.add)
            dist = pool.tile([TILE, T, K_GRAD], F32)
            S.activation(dist[:], ssq[:], AF.Sqrt, bias=eps_col[:])
            hsum = sc["den"]
            V.tensor_reduce(hsum[:], dist[:], axis=mybir.AxisListType.X, op=OP.add)
            hh = sc["c2d"]
            V.tensor_mul(hh[:], hsum[:], hsum[:])
            V.tensor_scalar(hh[:], hh[:], float(KERNEL_WIDTH ** 2 / (K_GRAD * K_GRAD)),
                            scalar2=float(EPS), op0=OP.mult, op1=OP.add)
            rhh = sc["rc2"]
            V.reciprocal(rhh[:], hh[:])
            dsq = tmpk
            V.tensor_mul(dsq[:], dist[:], dist[:])
            Ab = pool.tile([TILE, T, 6, K_GRAD], F32)
            warg = u  # reuse after? no -- u needed later. use separate
            warg = pool.tile([TILE, T, K_GRAD], F32, tag="warg")
            V.scalar_tensor_tensor(warg[:], dsq[:], -1.0, bc(rhh[:], (TILE, T, K_GRAD)),
                                   op0=OP.mult, op1=OP.mult)
            S.activation(Ab[:, :, 0, :], warg[:], AF.Exp)
            V.tensor_mul(Ab[:, :, 1, :], Ab[:, :, 0, :], u[:])
            V.tensor_mul(Ab[:, :, 2, :], Ab[:, :, 0, :], vv[:])
            V.tensor_mul(Ab[:, :, 3, :], Ab[:, :, 1, :], u[:])
            V.tensor_mul(Ab[:, :, 4, :], Ab[:, :, 1, :], vv[:])
            V.tensor_mul(Ab[:, :, 5, :], Ab[:, :, 2, :], vv[:])
            S6 = pool.tile([TILE, T, 6], F32)
            V.tensor_reduce(S6[:], Ab[:], axis=mybir.AxisListType.X, op=OP.add)
            # A = [[S0+reg, S1, S2], [S1, S3+reg, S4], [S2, S4, S5+reg]]
            for ch in (0, 3, 5):
                V.tensor_scalar(S6[:, :, ch], S6[:, :, ch], float(GRAD_REG),
                                scalar2=None, op0=OP.add)
            A00, A01, A02 = S6[:, :, 0], S6[:, :, 1], S6[:, :, 2]
            A11, A12, A22 = S6[:, :, 3], S6[:, :, 4], S6[:, :, 5]
            cof = {n: pool.tile([TILE, T], F32, tag=f"cof{n}", name=f"cof_{n}") for n in
                   "c00 c01 c02 c11 c12 c22 det".split()}

            def mulsub(out, a, b, c, d):
                V.tensor_mul(out, a, b)
                V.tensor_mul(sc["tmp1"][:], c, d)
                V.tensor_sub(out, out, sc["tmp1"][:])

            mulsub(cof["c00"][:], A11, A22, A12, A12)
            mulsub(cof["c01"][:], A02, A12, A01, A22)
            mulsub(cof["c02"][:], A01, A12, A02, A11)
            mulsub(cof["c11"][:], A00, A22, A02, A02)
            mulsub(cof["c12"][:], A01, A02, A00, A12)
            mulsub(cof["c22"][:], A00, A11, A01, A01)
            V.tensor_mul(cof["det"][:], A00, cof["c00"][:])
            V.tensor_mul(sc["tmp1"][:], A01, cof["c01"][:])
            V.tensor_add(cof["det"][:], cof["det"][:], sc["tmp1"][:])
            V.tensor_mul(sc["tmp1"][:], A02, cof["c02"][:])
            V.tensor_add(cof["det"][:], cof["det"][:], sc["tmp1"][:])
            rdet = sc["rden"]
            V.reciprocal(rdet[:], cof["det"][:])
            i10, i11, i12 = sc["d"], sc["dd"], sc["apq2"]
            i20, i22 = sc["t"], sc["t2"]
            V.tensor_mul(i10[:], cof["c01"][:], rdet[:])
            V.tensor_mul(i11[:], cof["c11"][:], rdet[:])
            V.tensor_mul(i12[:], cof["c12"][:], rdet[:])
            V.tensor_mul(i20[:], cof["c02"][:], rdet[:])
            V.tensor_mul(i22[:], cof["c22"][:], rdet[:])
            # i21 == i12

            gx = pool.tile([TILE, T, K_GRAD], F32)
            gy = pool.tile([TILE, T, K_GRAD], F32)
            for (g, ia, ib, ic) in ((gx, i10, i11, i12), (gy, i20, i12, i22)):
                V.tensor_mul(g[:], u[:], bc(ib[:], (TILE, T, K_GRAD)))
                V.tensor_mul(tmpk[:], vv[:], bc(ic[:], (TILE, T, K_GRAD)))
                V.tensor_add(g[:], g[:], tmpk[:])
                V.tensor_add(g[:], g[:], bc(ia[:], (TILE, T, K_GRAD)))
                V.tensor_mul(g[:], g[:], Ab[:, :, 0, :])
            nc.sync.dma_start(d_gx[:], gx[:])
            nc.sync.dma_start(d_gy[:], gy[:])

            # v_init = [sum_k gx*pn_d, sum_k gy*pn_d]
            vin = pool.tile([TILE, T, 6], F32)
            prodv = pool.tile([TILE, T, 3, K_GRAD], F32, tag="prodv")
            for (g, c0) in ((gx, 0), (gy, 3)):
                gb = g[:].unsqueeze(2).broadcast_to((TILE, T, 3, K_GRAD))
                V.tensor_mul(prodv[:], pn[:], gb)
                V.tensor_reduce(vin[:, :, c0:c0 + 3], prodv[:],
                                axis=mybir.AxisListType.X, op=OP.add)
            nc.sync.dma_start(d_vin[:], vin[:])
    nc.compile()
    return nc


def run_stencil(pos_s, nbr):
    """nbr: [N,20] sorted-space ids. Returns bas, gx, gy, vin in [N, ...] order."""
    nc = build_stencil_launch()
    in_maps = []
    for c in range(NCORES):
        own = slice(c * OWN, (c + 1) * OWN)
        # point (t, p) -> global c*OWN + t*128 + p ; layout [128 p, TPC t, ...]
        pi = pos_s[own].reshape(TPC, TILE, 3).transpose(1, 0, 2).copy()
        pnb = pos_s[nbr[own]].reshape(TPC, TILE, K_GRAD, 3).transpose(1, 0, 3, 2).copy()
        in_maps.append(dict(pi=np.ascontiguousarray(pi),
                            pn=np.ascontiguousarray(pnb)))
    res = run_launch(nc, in_maps, "stencil")
    outs = {}
    for name, ch in (("bas", 6), ("gx", K_GRAD), ("gy", K_GRAD), ("vin", 6)):
        full = np.zeros((NPTS, ch), np.float32)
        for c in range(NCORES):
            arr = res.results[c][name]  # [128, TPC, ch]
            full[c * OWN:(c + 1) * OWN] = arr.transpose(1, 0, 2).reshape(OWN, ch)
        outs[name] = full
    return outs


# ------------------------------------------------------------------
# launch 3: edge coeffs (R -> alpha/beta) + layer-0 scalar path -> x0
# ------------------------------------------------------------------

def build_x0_launch():
    nc = bacc.Bacc("TRN2", target_bir_lowering=False)
    T = TPC
    d_pi = nc.dram_tensor("pi", [TILE, T, 3], F32, kind="ExternalInput")
    d_pn = nc.dram_tensor("pn", [TILE, T, 3, K_GRAD], F32, kind="ExternalInput")
    d_vn = nc.dram_tensor("vn", [TILE, T, 6, K_GRAD], F32, kind="ExternalInput")
    d_bi = nc.dram_tensor("bi", [TILE, T, 6], F32, kind="ExternalInput")
    d_bn = nc.dram_tensor("bn", [TILE, T, 6, K_GRAD], F32, kind="ExternalInput")
    d_gx = nc.dram_tensor("gx", [TILE, T, K_GRAD], F32, kind="ExternalInput")
    d_gy = nc.dram_tensor("gy", [TILE, T, K_GRAD], F32, kind="ExternalInput")
    d_ws = nc.dram_tensor("ws", [9, 64], F32, kind="ExternalInput")
    d_bs = nc.dram_tensor("bs", [64, 1], F32, kind="ExternalInput")
    d_x0 = nc.dram_tensor("x0", [64, T * TILE], F32, kind="ExternalOutput")
    d_al = nc.dram_tensor("alpha", [TILE, T, K_GRAD], F32, kind="ExternalOutput")
    d_be = nc.dram_tensor("beta", [TILE, T, K_GRAD], F32, kind="ExternalOutput")

    V = nc.vector
    S = nc.scalar
    with TileContext(nc) as tc:
        with tc.tile_pool(name="sb", bufs=1) as pool, \
             tc.tile_pool(name="w2", bufs=2) as pool2, \
             tc.tile_pool(name="ps", bufs=2, space="PSUM") as psum:
            ident = pool.tile([TILE, TILE], F32)
            from concourse.masks import make_identity
            make_identity(nc, ident[:])
            pi = pool.tile([TILE, T, 3], F32)
            pn = pool.tile([TILE, T, 3, K_GRAD], F32)
            vn = pool.tile([TILE, T, 6, K_GRAD], F32)
            bi = pool.tile([TILE, T, 6], F32)
            bn = pool.tile([TILE, T, 6, K_GRAD], F32)
            gx = pool.tile([TILE, T, K_GRAD], F32)
            gy = pool.tile([TILE, T, K_GRAD], F32)
            ws = pool.tile([9, 64], F32)
            bs = pool.tile([64, 1], F32)
            for dst, s in ((pi, d_pi), (pn, d_pn), (vn, d_vn), (bi, d_bi),
                           (bn, d_bn), (gx, d_gx), (gy, d_gy), (ws, d_ws), (bs, d_bs)):
                nc.sync.dma_start(dst[:], s[:])

            def bc(ap, shape):
                while ap.ndim < len(shape):
                    ap = ap.unsqueeze(ap.ndim)
                return ap.broadcast_to(shape)

            KS = (TILE, T, K_GRAD)
            R = {}
            tmpk = pool.tile([TILE, T, K_GRAD], F32)
            for (name, a0, b0) in (("r00", 0, 0), ("r01", 0, 3), ("r10", 3, 0), ("r11", 3, 3)):
                r = pool.tile([TILE, T, K_GRAD], F32, name=f"R{name}")
                V.tensor_mul(r[:], bn[:, :, b0, :], bc(bi[:, :, a0], KS))
                V.tensor_mul(tmpk[:], bn[:, :, b0 + 1, :], bc(bi[:, :, a0 + 1], KS))
                V.tensor_add(r[:], r[:], tmpk[:])
                V.tensor_mul(tmpk[:], bn[:, :, b0 + 2, :], bc(bi[:, :, a0 + 2], KS))
                V.tensor_add(r[:], r[:], tmpk[:])
                R[name] = r
            al = pool.tile([TILE, T, K_GRAD], F32)
            be = pool.tile([TILE, T, K_GRAD], F32)
            V.tensor_mul(al[:], gx[:], R["r00"][:])
            V.tensor_mul(tmpk[:], gy[:], R["r10"][:])
            V.tensor_add(al[:], al[:], tmpk[:])
            V.tensor_mul(be[:], gx[:], R["r01"][:])
            V.tensor_mul(tmpk[:], gy[:], R["r11"][:])
            V.tensor_add(be[:], be[:], tmpk[:])
            nc.sync.dma_start(d_al[:], al[:])
            nc.sync.dma_start(d_be[:], be[:])

            cat = pool.tile([TILE, T, 9], F32)
            prod = pool.tile([TILE, T, 3, K_GRAD], F32)
            red = pool.tile([TILE, T, 3], F32)
            # xmax (centralized)
            rel = R["r00"]  # reuse [T,K] no -- need [T,3,K]; use prod
            V.tensor_sub(prod[:], pn[:], bc(pi[:], (TILE, T, 3, K_GRAD)))
            V.tensor_reduce(cat[:, :, 0:3], prod[:], axis=mybir.AxisListType.X, op=OP.max)
            # dv
            ab = bc(al[:].unsqueeze(2), (TILE, T, 3, K_GRAD))
            bb = bc(be[:].unsqueeze(2), (TILE, T, 3, K_GRAD))
            V.tensor_mul(prod[:], vn[:, :, 0:3, :], ab)
            V.tensor_reduce(cat[:, :, 3:6], prod[:], axis=mybir.AxisListType.X, op=OP.add)
            V.tensor_mul(prod[:], vn[:, :, 3:6, :], bb)
            V.tensor_reduce(red[:], prod[:], axis=mybir.AxisListType.X, op=OP.add)
            V.tensor_add(cat[:, :, 3:6], cat[:, :, 3:6], red[:])
            # cv
            V.tensor_mul(prod[:], vn[:, :, 0:3, :], bb)
            V.tensor_reduce(cat[:, :, 6:9], prod[:], axis=mybir.AxisListType.X, op=OP.add)
            V.tensor_mul(prod[:], vn[:, :, 3:6, :], ab)
            V.tensor_reduce(red[:], prod[:], axis=mybir.AxisListType.X, op=OP.add)
            V.tensor_sub(cat[:, :, 6:9], cat[:, :, 6:9], red[:])

            for t in range(T):
                ctp = psum.tile([9, TILE], F32, tag="ctp")
                nc.tensor.transpose(ctp[:], cat[:, t, :], ident[:])
                catT = pool2.tile([9, TILE], F32, tag="catT")
                S.copy(catT[:], ctp[:])
                xp = psum.tile([64, TILE], F32, tag="xp")
                nc.tensor.matmul(xp[:], ws[:], catT[:], start=True, stop=True)
                x0sb = pool2.tile([64, TILE], F32, tag="x0sb")
                S.activation(x0sb[:], xp[:], AF.Relu, bias=bs[:])
                nc.sync.dma_start(d_x0[:, t * TILE:(t + 1) * TILE], x0sb[:])
    nc.compile()
    return nc


def run_x0(pos_s, nbr, vin, bas, gxa, gya, Ws0, bs0):
    nc = build_x0_launch()
    in_maps = []
    for c in range(NCORES):
        own = slice(c * OWN, (c + 1) * OWN)
        g = np.arange(c * OWN, (c + 1) * OWN)

        def pm(arr_pts):  # [OWN, ...] -> [128, T, ...]
            return np.ascontiguousarray(
                arr_pts.reshape(TPC, TILE, *arr_pts.shape[1:]).swapaxes(0, 1))

        def pmk(arr_edges):  # [OWN, 20, ch] -> [128, T, ch, 20]
            a = arr_edges.reshape(TPC, TILE, K_GRAD, -1).transpose(1, 0, 3, 2)
            return np.ascontiguousarray(a)

        in_maps.append(dict(
            pi=pm(pos_s[own]), pn=pmk(pos_s[nbr[own]]),
            vn=pmk(vin[nbr[own]]), bi=pm(bas[own]), bn=pmk(bas[nbr[own]]),
            gx=pm(gxa[own]), gy=pm(gya[own]),
            ws=np.ascontiguousarray(Ws0), bs=np.ascontiguousarray(bs0[:, None])))
    res = run_launch(nc, in_maps, "x0")
    x0 = np.zeros((NPTS, 64), np.float32)
    al = np.zeros((NPTS, K_GRAD), np.float32)
    be = np.zeros((NPTS, K_GRAD), np.float32)
    for c in range(NCORES):
        own = slice(c * OWN, (c + 1) * OWN)
        x0[own] = res.results[c]["x0"].T
        al[own] = res.results[c]["alpha"].transpose(1, 0, 2).reshape(OWN, K_GRAD)
        be[own] = res.results[c]["beta"].transpose(1, 0, 2).reshape(OWN, K_GRAD)
    return x0, al, be


# ------------------------------------------------------------------
# v-layer launch (L4: v0, L6: v1): gv via SpMM, channel mix + norm gate
# ------------------------------------------------------------------

SLAB = 512  # per-tile slab rows (padded unique neighbors); adjusted at runtime
NKC = SLAB // TILE  # K chunks per slab


def _set_slab(n):
    global SLAB, NKC
    SLAB = int(math.ceil(n / TILE) * TILE)
    NKC = SLAB // TILE


def build_v_launch(Cf, Cvin, Cout):
    """gv = [WGx^T xs, WGy^T xs]; vo = cat(vprev, gv) @ Wv; norm-gate."""
    Ccat = Cvin + Cf
    nkc_cat = int(math.ceil(Ccat / TILE))
    nc = bacc.Bacc("TRN2", target_bir_lowering=False)
    T = TPC
    d_wgx = nc.dram_tensor("wgx", [TILE, T * NKC * TILE], F32, kind="ExternalInput")
    d_wgy = nc.dram_tensor("wgy", [TILE, T * NKC * TILE], F32, kind="ExternalInput")
    d_xs = nc.dram_tensor("xs", [TILE, T * NKC * Cf], F32, kind="ExternalInput")
    d_vp = nc.dram_tensor("vp", [TILE, T, 2, Cvin], F32, kind="ExternalInput")
    d_wv = nc.dram_tensor("wv", [TILE, nkc_cat * Cout], F32, kind="ExternalInput")
    d_bv = nc.dram_tensor("bv", [Cout, 1], F32, kind="ExternalInput")
    d_vo = nc.dram_tensor("vo", [2 * Cout, T * TILE], F32, kind="ExternalOutput")

    V = nc.vector
    S = nc.scalar
    with TileContext(nc) as tc:
        with tc.tile_pool(name="cst", bufs=1) as cpool, \
             tc.tile_pool(name="big", bufs=3) as bpool, \
             tc.tile_pool(name="wk", bufs=4) as pool, \
             tc.tile_pool(name="ps", bufs=1, space="PSUM") as psum:
            ident = cpool.tile([TILE, TILE], F32)
            from concourse.masks import make_identity
            make_identity(nc, ident[:])
            onesc = cpool.tile([Cout, 1], F32)
            V.memset(onesc[:], 1.0)
            wv = cpool.tile([TILE, nkc_cat * Cout], F32)
            nc.sync.dma_start(wv[:], d_wv[:])
            bv = cpool.tile([Cout, 1], F32)
            nc.sync.dma_start(bv[:], d_bv[:])
            vp = cpool.tile([TILE, T, 2, Cvin], F32)
            nc.sync.dma_start(vp[:], d_vp[:])
            eps_row = cpool.tile([1, 1], F32)
            V.memset(eps_row[:], float(EPS))

            for t in range(T):
                wgx_t = bpool.tile([TILE, NKC * TILE], F32, tag="wgx")
                wgy_t = bpool.tile([TILE, NKC * TILE], F32, tag="wgy")
                xs_t = bpool.tile([TILE, NKC * Cf], F32, tag="xs")
                nc.sync.dma_start(wgx_t[:], d_wgx[:, t * NKC * TILE:(t + 1) * NKC * TILE])
                nc.sync.dma_start(wgy_t[:], d_wgy[:, t * NKC * TILE:(t + 1) * NKC * TILE])
                nc.sync.dma_start(xs_t[:], d_xs[:, t * NKC * Cf:(t + 1) * NKC * Cf])

                vo_sb = []
                for a, wg_t in ((0, wgx_t), (1, wgy_t)):
                    gvp = psum.tile([TILE, Cf], F32, tag="gvp")
                    for kc in range(NKC):
                        nc.tensor.matmul(gvp[:], wg_t[:, kc * TILE:(kc + 1) * TILE],
                                         xs_t[:, kc * Cf:(kc + 1) * Cf],
                                         start=(kc == 0), stop=(kc == NKC - 1))
                    cat_a = pool.tile([TILE, Ccat], F32, tag="cat")
                    V.tensor_copy(cat_a[:, 0:Cvin], vp[:, t, a, :])
                    S.copy(cat_a[:, Cvin:], gvp[:])
                    vop = psum.tile([Cout, TILE], F32, tag="vop")
                    catTs = []
                    for kc in range(nkc_cat):
                        kl = min(TILE, Ccat - kc * TILE)
                        ctp = psum.tile([TILE, TILE], F32, tag="ctp", bufs=2)
                        nc.tensor.transpose(ctp[:kl, :], cat_a[:, kc * TILE:kc * TILE + kl],
                                            ident[:])
                        catT = pool.tile([TILE, TILE], F32, tag=f"catT{kc}")
                        S.copy(catT[:kl, :], ctp[:kl, :])
                        catTs.append((catT, kl))
                    for kc in range(nkc_cat):
                        catT, kl = catTs[kc]
                        nc.tensor.matmul(vop[:], wv[:kl, kc * Cout:(kc + 1) * Cout],
                                         catT[:kl, :], start=(kc == 0), stop=(kc == nkc_cat - 1))
                    vo_a = pool.tile([Cout, TILE], F32, tag="voa")
                    S.copy(vo_a[:], vop[:])
                    vo_sb.append(vo_a)

                sq0 = pool.tile([Cout, TILE], F32, tag="sq0")
                sq1 = pool.tile([Cout, TILE], F32, tag="sq1")
                V.tensor_mul(sq0[:], vo_sb[0][:], vo_sb[0][:])
                V.tensor_mul(sq1[:], vo_sb[1][:], vo_sb[1][:])
                n2p = psum.tile([1, TILE], F32, tag="n2p")
                nc.tensor.matmul(n2p[:], onesc[:], sq0[:], start=True, stop=False)
                nc.tensor.matmul(n2p[:], onesc[:], sq1[:], start=False, stop=True)
                nrow = pool.tile([1, 2 * TILE], F32, tag="nrow")
                S.activation(nrow[:, 0:TILE], n2p[:], AF.Sqrt, bias=eps_row[:])
                npe = pool.tile([1, TILE], F32, tag="npe")
                V.tensor_scalar(npe[:], nrow[:, 0:TILE], float(EPS), scalar2=None, op0=OP.add)
                V.reciprocal(nrow[:, TILE:], npe[:])
                bcp = psum.tile([Cout, 2 * TILE], F32, tag="bcp")
                ones1 = cpool.tile([1, Cout], F32)
                V.memset(ones1[:], 1.0)
                nc.tensor.matmul(bcp[:], ones1[:], nrow[:], start=True, stop=True)
                gate = pool.tile([Cout, TILE], F32, tag="gate")
                S.activation(gate[:], bcp[:, 0:TILE], AF.Relu, bias=bv[:])
                V.tensor_mul(gate[:], gate[:], bcp[:, TILE:])
                V.tensor_mul(vo_sb[0][:], vo_sb[0][:], gate[:])
                V.tensor_mul(vo_sb[1][:], vo_sb[1][:], gate[:])
                nc.sync.dma_start(d_vo[0:Cout, t * TILE:(t + 1) * TILE], vo_sb[0][:])
                nc.sync.dma_start(d_vo[Cout:, t * TILE:(t + 1) * TILE], vo_sb[1][:])
    nc.compile()
    return nc


def make_slab_plan(nbr):
    """Per tile: sorted unique neighbor ids padded to SLAB; and edge->slot map."""
    uniq = [np.unique(nbr[t * TILE:(t + 1) * TILE]) for t in range(NTILES)]
    _set_slab(max(max(len(u) for u in uniq), 256))
    slabs = []
    slots = np.zeros((NPTS, K_GRAD), np.int32)
    for t in range(NTILES):
        rows = slice(t * TILE, (t + 1) * TILE)
        u = uniq[t]
        pad = np.full(SLAB, u[0], np.int64)
        pad[:len(u)] = u
        slabs.append(pad)
        lookup = {int(j): i for i, j in enumerate(u)}
        s = np.array([[lookup[int(j)] for j in row] for row in nbr[rows]], np.int32)
        slots[rows] = s
    return np.array(slabs), slots


def build_W(vals, slots):
    """vals [OWN_rows, 20] for one core's tiles -> dense [128, T*NKC*128]."""
    T = vals.shape[0] // TILE
    W = np.zeros((T, SLAB, TILE), np.float32)
    for t in range(T):
        for p in range(TILE):
            W[t, slots[t * TILE + p], p] = vals[t * TILE + p]
    # [T, SLAB, 128] -> chunks [T, NKC, 128, 128] -> [128, T*NKC*128]
    W = W.reshape(T, NKC, TILE, TILE).transpose(2, 0, 1, 3).reshape(TILE, T * NKC * TILE)
    return np.ascontiguousarray(W)


def build_slab_feat(feat, slabs_core):
    """feat [NPTS, C]; slabs_core [T, SLAB] -> [128, T*NKC*C]."""
    T = slabs_core.shape[0]
    C = feat.shape[1]
    s = feat[slabs_core]          # [T, SLAB, C]
    s = s.reshape(T, NKC, TILE, C).transpose(2, 0, 1, 3).reshape(TILE, T * NKC * C)
    return np.ascontiguousarray(s)


def run_vlayer(name, Cf, Cvin, Cout, gxa, gya, slots, slabs, xfeat, vprev, Wv, bvv):
    nc = build_v_launch(Cf, Cvin, Cout)
    Ccat = Cvin + Cf
    nkc_cat = int(math.ceil(Ccat / TILE))
    wv_np = np.zeros((TILE, nkc_cat * Cout), np.float32)
    for kc in range(nkc_cat):
        kl = min(TILE, Ccat - kc * TILE)
        wv_np[:kl, kc * Cout:(kc + 1) * Cout] = Wv[kc * TILE:kc * TILE + kl, :]
    in_maps = []
    for c in range(NCORES):
        own = slice(c * OWN, (c + 1) * OWN)
        tsl = slice(c * TPC, (c + 1) * TPC)
        in_maps.append(dict(
            wgx=build_W(gxa[own], slots[own]),
            wgy=build_W(gya[own], slots[own]),
            xs=build_slab_feat(xfeat, slabs[tsl]),
            vp=np.ascontiguousarray(
                vprev[own].reshape(TPC, TILE, 2, Cvin).swapaxes(0, 1)),
            wv=wv_np, bv=np.ascontiguousarray(bvv[:, None])))
    res = run_launch(nc, in_maps, name)
    vo = np.zeros((NPTS, 2, Cout), np.float32)
    for c in range(NCORES):
        own = slice(c * OWN, (c + 1) * OWN)
        arr = res.results[c]["vo"]  # [2*Cout, OWN]
        vo[own, 0] = arr[:Cout].T
        vo[own, 1] = arr[Cout:].T
    return vo.reshape(NPTS, 2 * Cout)


# ------------------------------------------------------------------
# x-layer launch (L5: x1, L7: x2): dv/cv via SpMM, xmax via edge reduce
# ------------------------------------------------------------------

def build_x_launch(C, Cout):
    """x_out = relu(cat(xmax, dv, cv) @ Ws + bs); C = input feature ch."""
    Ccat = 3 * C
    nkc_cat = int(math.ceil(Ccat / TILE))
    nmc = int(math.ceil(Cout / TILE))  # output column chunks
    nc = bacc.Bacc("TRN2", target_bir_lowering=False)
    T = TPC
    d_wa = nc.dram_tensor("wa", [TILE, T * NKC * TILE], F32, kind="ExternalInput")
    d_wb = nc.dram_tensor("wb", [TILE, T * NKC * TILE], F32, kind="ExternalInput")
    d_v0s = nc.dram_tensor("v0s", [TILE, T * NKC * C], F32, kind="ExternalInput")
    d_v1s = nc.dram_tensor("v1s", [TILE, T * NKC * C], F32, kind="ExternalInput")
    d_xn = nc.dram_tensor("xn", [TILE, T, C, K_GRAD], F32, kind="ExternalInput")
    d_ws = nc.dram_tensor("ws", [TILE, nkc_cat * Cout], F32, kind="ExternalInput")
    d_bs = nc.dram_tensor("bs", [TILE, nmc], F32, kind="ExternalInput")
    d_xo = nc.dram_tensor("xo", [Cout, T * TILE], F32, kind="ExternalOutput")

    V = nc.vector
    S = nc.scalar
    with TileContext(nc) as tc:
        with tc.tile_pool(name="cst", bufs=1) as cpool, \
             tc.tile_pool(name="big", bufs=3) as bpool, \
             tc.tile_pool(name="wk", bufs=4) as pool, \
             tc.tile_pool(name="ps", bufs=1, space="PSUM") as psum:
            ident = cpool.tile([TILE, TILE], F32)
            from concourse.masks import make_identity
            make_identity(nc, ident[:])
            ws = cpool.tile([TILE, nkc_cat * Cout], F32)
            nc.sync.dma_start(ws[:], d_ws[:])
            bs = cpool.tile([TILE, nmc], F32)
            nc.sync.dma_start(bs[:], d_bs[:])

            for t in range(T):
                wa_t = bpool.tile([TILE, NKC * TILE], F32, tag="wa")
                wb_t = bpool.tile([TILE, NKC * TILE], F32, tag="wb")
                v0_t = bpool.tile([TILE, NKC * C], F32, tag="v0")
                v1_t = bpool.tile([TILE, NKC * C], F32, tag="v1")
                xn_t = bpool.tile([TILE, C, K_GRAD], F32, tag="xn")
                nc.sync.dma_start(wa_t[:], d_wa[:, t * NKC * TILE:(t + 1) * NKC * TILE])
                nc.sync.dma_start(wb_t[:], d_wb[:, t * NKC * TILE:(t + 1) * NKC * TILE])
                nc.sync.dma_start(v0_t[:], d_v0s[:, t * NKC * C:(t + 1) * NKC * C])
                nc.sync.dma_start(v1_t[:], d_v1s[:, t * NKC * C:(t + 1) * NKC * C])
                nc.sync.dma_start(xn_t[:], d_xn[:, t, :, :])

                cat = pool.tile([TILE, Ccat], F32, tag="cat")
                V.tensor_reduce(cat[:, 0:C], xn_t[:], axis=mybir.AxisListType.X, op=OP.max)
                # dv = Wa^T v0s + Wb^T v1s
                dvp = psum.tile([TILE, C], F32, tag="dvp")
                for kc in range(NKC):
                    nc.tensor.matmul(dvp[:], wa_t[:, kc * TILE:(kc + 1) * TILE],
                                     v0_t[:, kc * C:(kc + 1) * C],
                                     start=(kc == 0), stop=False)
                for kc in range(NKC):
                    nc.tensor.matmul(dvp[:], wb_t[:, kc * TILE:(kc + 1) * TILE],
                                     v1_t[:, kc * C:(kc + 1) * C],
                                     start=False, stop=(kc == NKC - 1))
                S.copy(cat[:, C:2 * C], dvp[:])
                # cv = Wb^T v0s - Wa^T v1s
                p1 = psum.tile([TILE, C], F32, tag="p1")
                for kc in range(NKC):
                    nc.tensor.matmul(p1[:], wb_t[:, kc * TILE:(kc + 1) * TILE],
                                     v0_t[:, kc * C:(kc + 1) * C],
                                     start=(kc == 0), stop=(kc == NKC - 1))
                p2 = psum.tile([TILE, C], F32, tag="p2")
                for kc in range(NKC):
                    nc.tensor.matmul(p2[:], wa_t[:, kc * TILE:(kc + 1) * TILE],
                                     v1_t[:, kc * C:(kc + 1) * C],
                                     start=(kc == 0), stop=(kc == NKC - 1))
                S.copy(cat[:, 2 * C:], p1[:])
                V.tensor_sub(cat[:, 2 * C:], cat[:, 2 * C:], p2[:])

                # transpose cat -> matmul Ws chunks
                catTs = []
                for kc in range(nkc_cat):
                    kl = min(TILE, Ccat - kc * TILE)
                    ctp = psum.tile([TILE, TILE], F32, tag="ctp", bufs=2)
                    nc.tensor.transpose(ctp[:kl, :], cat[:, kc * TILE:kc * TILE + kl],
                                        ident[:])
                    catT = pool.tile([TILE, TILE], F32, tag=f"catT{kc}")
                    S.copy(catT[:kl, :], ctp[:kl, :])
                    catTs.append((catT, kl))
                for mc in range(nmc):
                    ml = min(TILE, Cout - mc * TILE)
                    xop = psum.tile([TILE, TILE], F32, tag="xop")
                    for kc in range(nkc_cat):
                        catT, kl = catTs[kc]
                        nc.tensor.matmul(
                            xop[:ml, :], ws[:kl, kc * Cout + mc * TILE: kc * Cout + mc * TILE + ml],
                            catT[:kl, :], start=(kc == 0), stop=(kc == nkc_cat - 1))
                    xo_sb = pool.tile([TILE, TILE], F32, tag="xosb")
                    S.activation(xo_sb[:ml, :], xop[:ml, :], AF.Relu,
                                 bias=bs[0:ml, mc:mc + 1])
                    nc.sync.dma_start(d_xo[mc * TILE:mc * TILE + ml, t * TILE:(t + 1) * TILE],
                                      xo_sb[:ml, :])
    nc.compile()
    return nc


def run_xlayer(name, C, Cout, al, be, slots, slabs, v_feat, x_feat, nbr, Ws, bsv):
    """v_feat [NPTS, 2, C]; x_feat [NPTS, C] (for xmax); returns x_out [NPTS, Cout]."""
    nc = build_x_launch(C, Cout)
    Ccat = 3 * C
    nkc_cat = int(math.ceil(Ccat / TILE))
    nmc = int(math.ceil(Cout / TILE))
    ws_np = np.zeros((TILE, nkc_cat * Cout), np.float32)
    for kc in range(nkc_cat):
        kl = min(TILE, Ccat - kc * TILE)
        ws_np[:kl, kc * Cout:(kc + 1) * Cout] = Ws[kc * TILE:kc * TILE + kl, :]
    bs_np = np.zeros((TILE, nmc), np.float32)
    for mc in range(nmc):
        ml = min(TILE, Cout - mc * TILE)
        bs_np[:ml, mc] = bsv[mc * TILE:mc * TILE + ml]
    in_maps = []
    for c in range(NCORES):
        own = slice(c * OWN, (c + 1) * OWN)
        tsl = slice(c * TPC, (c + 1) * TPC)
        xn = x_feat[nbr[own]].reshape(TPC, TILE, K_GRAD, C).transpose(1, 0, 3, 2)
        in_maps.append(dict(
            wa=build_W(al[own], slots[own]),
            wb=build_W(be[own], slots[own]),
            v0s=build_slab_feat(v_feat[:, 0], slabs[tsl]),
            v1s=build_slab_feat(v_feat[:, 1], slabs[tsl]),
            xn=np.ascontiguousarray(xn),
            ws=ws_np, bs=bs_np))
    res = run_launch(nc, in_maps, name)
    xo = np.zeros((NPTS, Cout), np.float32)
    for c in range(NCORES):
        own = slice(c * OWN, (c + 1) * OWN)
        xo[own] = res.results[c]["xo"].T
    return xo


def kernel(points, als_ppoints, Ws0, bs0, Wv0, bv0, Ws1, bs1, Wv1, bv1, Ws2, bs2):
    pos = np.concatenate([np.asarray(points), np.asarray(als_ppoints)], 0).astype(np.float32)
    Ws0, bs0, Wv0, bv0 = (np.asarray(a, np.float32) for a in (Ws0, bs0, Wv0, bv0))
    Ws1, bs1, Wv1, bv1 = (np.asarray(a, np.float32) for a in (Ws1, bs1, Wv1, bv1))
    Ws2, bs2 = np.asarray(Ws2, np.float32), np.asarray(bs2, np.float32)

    perm, nbr = plan_and_knn(pos)
    pos_s = pos[perm].astype(np.float32)
    st = run_stencil(pos_s, nbr)
    x0, al, be = run_x0(pos_s, nbr, st["vin"], st["bas"], st["gx"], st["gy"], Ws0, bs0)
    slabs, slots = make_slab_plan(nbr)
    v0 = run_vlayer("v0", 64, 3, 64, st["gx"], st["gy"], slots, slabs,
                    x0, st["vin"], Wv0, bv0)
    x1 = run_xlayer("x1", 64, 128, al, be, slots, slabs,
                    v0.reshape(NPTS, 2, 64), x0, nbr, Ws1, bs1)
    v1 = run_vlayer("v1", 128, 64, 128, st["gx"], st["gy"], slots, slabs,
                    x1, v0, Wv1, bv1)
    x2 = run_xlayer("x2", 128, 256, al, be, slots, slabs,
                    v1.reshape(NPTS, 2, 128), x1, nbr, Ws2, bs2)
    inv = np.empty_like(perm)
    inv[perm] = np.arange(NPTS)
    return (x0[inv], x1[inv], x2[inv])
